# revision 75
# baseline (speedup 1.0000x reference)
"""CorrelationFusion Trainium2 kernel.

Per-clip math (T=8 frames, G=4 groups, 3x3 correlation window):
  corr[g, tt*9+ij, p] = sum_cp x[tt, g*64+cp, p] * xpad[tt+1, g*64+cp, p+d(ij)]
  wx[g, o*8+t, p]     = sum_i conv_w[g, o*8+t, i]*corr[g, i, p] + conv_b[g, o*8+t]
  out[o, g*64+cp, p]  = sum_t wx'[g, o*8+t, p] * x[t, cp*4+g, p]
  (wx' = wx + 1 on the t==o rows -- the residual folded into the conv bias)

Mapping:
  - per-pixel products on VectorE in bf16 (2x mode), channels on partitions;
    corr products merged into di-trios (one DVE op per (tt, dj) via an
    overlapping 3-row-window AP), phase2 products one op per (g, o)
  - partition reductions via TensorE matmuls, 4-way COLUMN-TILED
    (tile_position col groups) so 4 small-M matmuls run concurrently
  - the 1x1 grouped conv is a block-diagonal matmul over the 63 corr rows
    (k-rows permuted to match the col-tiled corr psum layout)
  - frames DMA'd contiguously into flat staging tiles ('a' operands and the
    center 'b' read those directly); padded P/S copies + edge replication
    on ScalarE (strided DMA writes measured 4x slower -- don't)
  - wx rows replicated into the (cp16, t8)-interleaved layout via a
    broadcast-read DMA from a DRAM bounce buffer
  - corr / conv weights / output in bf16 (rel-err budget has 4x margin);
    host converts the bf16 output back to fp32
  - data-parallel over the 8 clips: one clip per NeuronCore
"""

import numpy as np
import ml_dtypes

T = 8
TO = 8
G = 4
CPG = 64
C = 256
H = 56
W = 56
PIX = H * W
NCORES = 8
PH = 58   # padded tile rows
PW = 60   # padded tile cols (extra pad for 4B alignment of bf16 rows)
NCH = 7   # pixel chunks per image
CHW = 8   # rows per chunk
CHN = CHW * W  # 448 pixels per chunk
_CACHE = {}


def _corr_row(i, gh):
    """psum/partition row of corr index i (0..62) for channel-half gh."""
    return 32 * (i % 4) + 2 * (i // 4) + gh


def _exec_seq():
    """corr product execution order: tt=0 specially ordered for the head,
    tt>=1 grouped in di-trios per dj."""
    seq = [4, 1, 7, 0, 3, 6, 2, 5, 8]
    for tt in range(1, T - 1):
        for dj in (-1, 0, 1):
            seq += [tt * 9 + k * 3 + (dj + 1) for k in range(3)]
    return seq


_SEQ = _exec_seq()
_FIRSTS, _LASTS = {}, {}
for _r in _SEQ:
    _FIRSTS.setdefault(_r % 4, _r)
    _LASTS[_r % 4] = _r
FIRST_RS = frozenset(_FIRSTS.values())
LAST_RS = frozenset(_LASTS.values())


def _build_module(loop_k=1):
    import contextlib

    import concourse.bass as bass
    import concourse.bacc as bacc
    import concourse.mybir as mybir
    import concourse.tile as tile

    fp32 = mybir.dt.float32
    bf16 = mybir.dt.bfloat16

    nc = bacc.Bacc(name="corrfusion")
    xin = nc.dram_tensor("xin", [T, C, H, W], bf16, kind="ExternalInput")
    wf2 = nc.dram_tensor("wf2", [128, 2, 128], bf16, kind="ExternalInput")
    bm2 = nc.dram_tensor("bm2", [128, 96], bf16, kind="ExternalInput")
    tones = nc.dram_tensor("tones", [128, 4, 16], bf16, kind="ExternalInput")
    bvec = nc.dram_tensor("bvec", [128, 2], fp32, kind="ExternalInput")
    out = nc.dram_tensor("out", [TO, C, H, W], bf16, kind="ExternalOutput")

    xin_base = xin[:, :, :, :]                                 # base AP for manual APs
    out_r = out.rearrange("o (g cpc k) h w -> o g cpc k (h w)", g=4, cpc=4, k=16)

    with tile.TileContext(nc) as tc:
        with tc.tile_pool(name="consts", bufs=1) as consts, \
             tc.tile_pool(name="corrbuf", bufs=1) as corrbuf, \
             tc.tile_pool(name="xt", bufs=2) as xtp, \
             tc.tile_pool(name="wxdp", bufs=1, space="DRAM") as wxdp, \
             tc.tile_pool(name="psum", bufs=1, space="PSUM") as psum:

            # consts on the gpsimd queue: scalar must be free for the
            # first frame-pad copies, sync for the first stage loads
            wf_sb = consts.tile([128, 2, 128], bf16)
            nc.gpsimd.dma_start(out=wf_sb, in_=wf2[:, :, :])
            bm_sb = consts.tile([128, 96], bf16)
            nc.gpsimd.dma_start(out=bm_sb, in_=bm2[:, :])
            to_sb = consts.tile([128, 4, 16], bf16)
            nc.gpsimd.dma_start(out=to_sb, in_=tones[:, :, :])
            bv_sb = consts.tile([128, 2], fp32)
            nc.gpsimd.dma_start(out=bv_sb, in_=bvec[:, :])

            # shared tag: corr_sb[0] is dead once conv(0)'s matmuls have
            # read it, well before corr(1)'s drains write the buffer
            corr_sb = [
                corrbuf.tile([128, PIX], bf16, tag="corr", name=f"corr{i}")
                for i in range(2)
            ]
            # one shared staging buffer: wx_sb[gp] is dead once the wxd
            # bounce DMA has read it, so the two conv phases can share
            wx_sb = [
                corrbuf.tile([128, PIX], bf16, tag="wx", name=f"wx{i}")
                for i in range(2)
            ]
            loop_cm = (
                tc.For_i(0, loop_k, 1) if loop_k > 1 else contextlib.nullcontext()
            )
            with loop_cm:
                _build_body(nc, tc, tile, bass, mybir, fp32, bf16, xin, xin_base,
                            out_r, wf_sb, bm_sb, to_sb, bv_sb, corr_sb, wx_sb,
                            xtp, wxdp, psum)
    nc.compile()
    return nc


def _build_body(nc, tc, tile, bass, mybir, fp32, bf16, xin, xin_base, out_r,
                wf_sb, bm_sb, to_sb, bv_sb, corr_sb, wx_sb, xtp, wxdp, psum):
    # rows 126/127 (unused by the col-tiled corr layout) must read as zeros
    # in the conv matmul; memset the whole 32-row group (engines need a
    # 32-aligned partition base), the drains overwrite rows 96..125
    # (gpsimd: vector/scalar must stay clear for the first frame work)
    for i in range(2):
        nc.gpsimd.memset(corr_sb[i][96:128, :], 0.0)

    wxd = [None, None]

    frames_pools = {}  # set by the caller: frames, stage, prods
    ptiles = {0: {}, 1: {}}
    stiles = {0: {}, 1: {}}
    sgtiles = {0: {}, 1: {}}

    def load_stg(ct, t, split=False):
        frames, stage, prods = frames_pools["pools"]
        cs = ct * 128
        stg = stage.tile([128, PIX], bf16, tag="fstage", name=f"stg{ct}_{t}")
        if split:
            # pixel-half DMAs: the first (center) product can start on the
            # first half while the second is still in flight
            hp = 4 * CHN
            nc.sync.dma_start(out=stg[:, 0:hp], in_=xin[t, cs:cs + 128, 0:32, :])
            nc.sync.dma_start(out=stg[:, hp:], in_=xin[t, cs:cs + 128, 32:56, :])
        else:
            nc.sync.dma_start(out=stg, in_=xin[t, cs:cs + 128, :, :])
        sgtiles[ct][t] = stg
        return stg

    def load_frame(ct, t, warmup=False, stg=None):
        frames, stage, prods = frames_pools["pools"]
        ptile, stile, sgtile = ptiles[ct], stiles[ct], sgtiles[ct]
        # contiguous DMA into a staging tile; 'a' operands and the center
        # (di=0,dj=0) 'b' operand read the flat staging tile directly, so
        # the padded-copy latency is off the product critical path
        if stg is None:
            stg = load_stg(ct, t)
        stg3 = stg.rearrange("p (h w) -> p h w", h=H)
        if t > 0:
            # frame 0 is only ever the 'a' operand: no padded tiles needed.
            # P serves the di-shifted dj=0 'b' reads (rows padded).
            # warmup (very first pair): vector is idle until the first
            # stage tiles land, so it carries the P work itself
            pcopy = nc.vector.tensor_copy if warmup else nc.scalar.copy
            P = frames.tile([128, PH, PW], bf16, tag=f"P{t % 4}", name=f"P{ct}_{t}")
            if warmup:
                # row-halves matching the split head DMAs: the first copy
                # starts as soon as the first half-DMA lands
                pcopy(P[:, 1:33, 2:58], stg3[:, 0:32, :])
                pcopy(P[:, 33:57, 2:58], stg3[:, 32:56, :])
            else:
                pcopy(P[:, 1:57, 2:58], stg3)
            pcopy(P[:, 0:1, 2:58], P[:, 1:2, 2:58])
            pcopy(P[:, 57:58, 2:58], P[:, 56:57, 2:58])
            ptile[t] = P
            # S always on scalar: with the split head DMAs its chain
            # finishes before the first S-dependent trio needs it, and the
            # vector queue keeps those ~2us for products
            S = frames.tile([128, PH, PW], bf16, tag=f"S{t % 4}", name=f"S{ct}_{t}")
            scopy = nc.scalar.copy
            if warmup:
                scopy(S[:, 1:33, 1:57], stg3[:, 0:32, :])
                scopy(S[:, 33:57, 1:57], stg3[:, 32:56, :])
            else:
                scopy(S[:, 1:57, 1:57], stg3)
            # S is read at cols 0:56 / 2:58 (dj=-1/+1): col pads 0 and 57
            scopy(S[:, 1:57, 0:1], S[:, 1:57, 1:2])
            scopy(S[:, 1:57, 57:58], S[:, 1:57, 56:57])
            scopy(S[:, 0:1, 0:58], S[:, 1:2, 0:58])
            scopy(S[:, 57:58, 0:58], S[:, 56:57, 0:58])
            stile[t] = S

    def corr_phase(ct):
        if True:
            frames, stage, prods = frames_pools["pools"]
            cps = [
                psum.tile([128, CHN], fp32, tag=f"b{c}", name=f"cps{ct}_{c}")
                for c in range(NCH)
            ]
            ptile = ptiles[ct]
            stile = stiles[ct]
            sgtile = sgtiles[ct]

            if ct == 0 and 0 not in sgtile and 1 not in sgtile:
                # both head frames, half-DMAs interleaved across frames so
                # the first halves of BOTH land first and the center
                # product starts at ~half the total flight time.
                # (padded copies are emitted after the first product.)
                frames_, stage_, _ = frames_pools["pools"]
                hp = 4 * CHN
                stgs = []
                for t in (0, 1):
                    stgs.append(
                        stage_.tile([128, PIX], bf16, tag="fstage", name=f"stg0_{t}")
                    )
                    sgtile[t] = stgs[t]
                for half in range(2):
                    s, e = (0, hp) if half == 0 else (hp, PIX)
                    r0, r1 = (0, 32) if half == 0 else (32, 56)
                    for t in (0, 1):
                        nc.sync.dma_start(
                            out=stgs[t][:, s:e], in_=xin[t, 0:128, r0:r1, :]
                        )
            if 0 not in sgtile:
                load_frame(ct, 0)
            if 1 not in sgtile:
                load_frame(ct, 1)
            def emit_reduce(r, pr2d):
                # col-tiled reduction: product r -> col group r%4,
                # rows 2*(r//4)+gh; lhsT = 32-col slice of the shifted
                # ones matrix.  start/stop from the precomputed execution
                # order (trio grouping reorders products within a tt)
                cg = r % 4
                q = r // 4
                lhsT = bm_sb[:, 62 - 2 * q:94 - 2 * q]
                for c in range(NCH):
                    nc.tensor.matmul(
                        cps[c][32 * cg:32 * cg + 32, :],
                        lhsT,
                        pr2d[:, c * CHN:(c + 1) * CHN],
                        start=(r in FIRST_RS),
                        stop=(r in LAST_RS),
                        tile_position=(0, 32 * cg),
                    )

            def b_win(tt, dj, n):
                # overlapping di-window AP: [128, n, 56, 56], di stride =
                # one padded row (60); n=3 covers di in {-1,0,1}, n=2 the
                # duo {-1,+1} (stride 2 rows)
                tile_ap = ptile[tt + 1] if dj == 0 else stile[tt + 1]
                col = 2 if dj >= 0 else 0
                return bass.AP(
                    tensor=tile_ap.tensor,
                    offset=tile_ap.offset + col,
                    ap=[[PH * PW, 128], [PW * (4 - n), n], [PW, 56], [1, 56]],
                )

            for tt in range(T - 1):
                if tt + 2 < T and not (ct == 0 and tt == 0):
                    load_frame(ct, tt + 2)
                if ct == 0 and tt == T - 2:
                    # preload next half's first frames while this half's
                    # tail products still run (keeps the scalar queue from
                    # serializing drains ahead of them)
                    load_frame(1, 0)
                    load_frame(1, 1)
                a3 = sgtile[tt].rearrange("p (h w) -> p h w", h=H)
                if tt == 0:
                    # center product split in pixel halves (reads only the
                    # flat half-loaded stage tiles) leads the vector queue
                    prc = prods.tile([128, 3, PIX], bf16, tag="ptrio", name="prc")
                    HP = 4 * CHN
                    nc.vector.tensor_mul(
                        prc[:, 0, 0:HP], sgtile[0][:, 0:HP], sgtile[1][:, 0:HP]
                    )
                    if ct == 0 and 1 not in ptile:
                        # first padded tiles right behind the first product;
                        # frame 2 deferred so the scheduler can't slot its
                        # big copies ahead of S1's pads on the scalar queue
                        load_frame(0, 1, warmup=True, stg=sgtile[1])
                    nc.vector.tensor_mul(
                        prc[:, 0, HP:], sgtile[0][:, HP:], sgtile[1][:, HP:]
                    )
                    emit_reduce(tt * 9 + 4, prc[:, 0, :])
                    # dj=0 duo (di = -1,+1), then the two S trios
                    duo = prc[:, 1:3, :].rearrange("p d (h w) -> p d h w", h=H)
                    a2 = a3.unsqueeze(1).broadcast_to((128, 2, H, W))
                    nc.vector.tensor_mul(duo, a2, b_win(tt, 0, 2))
                    emit_reduce(tt * 9 + 1, prc[:, 1, :])
                    emit_reduce(tt * 9 + 7, prc[:, 2, :])
                    dj_iter = (-1, 1)
                else:
                    dj_iter = (-1, 0, 1)
                a3b = a3.unsqueeze(1).broadcast_to((128, 3, H, W))
                for dj in dj_iter:
                    pr = prods.tile([128, 3, PIX], bf16, tag="ptrio", name="pr")
                    pr4d = pr.rearrange("p d (h w) -> p d h w", h=H)
                    nc.vector.tensor_mul(pr4d, a3b, b_win(tt, dj, 3))
                    for k in range(3):
                        emit_reduce(tt * 9 + k * 3 + (dj + 1), pr[:, k, :])
                if ct == 0 and tt == 0 and 2 not in sgtile:
                    load_frame(0, 2)
            for c in range(NCH):
                nc.scalar.copy(
                    corr_sb[ct][0:126, c * CHN:(c + 1) * CHN],
                    cps[c][0:126, :],
                )

    def conv_phase(gp):
        # grouped 1x1 conv (+bias +residual); bounce wx to DRAM so the
        # per-(o,g) replication is one broadcast-read DMA
        for c in range(NCH):
            wpp = psum.tile([128, CHN], fp32, tag="wp", name=f"wpp{gp}_{c}")
            nc.tensor.matmul(
                wpp,
                wf_sb[:, gp, :],
                corr_sb[gp][:, c * CHN:(c + 1) * CHN],
                start=True,
                stop=True,
            )
            nc.scalar.activation(
                wx_sb[gp][:, c * CHN:(c + 1) * CHN],
                wpp,
                mybir.ActivationFunctionType.Identity,
                bias=bv_sb[:, gp:gp + 1],
                scale=1.0,
            )
        # trigger on the scalar queue: its dep (the wx activations) sits
        # right before it there, so it never blocks a queue head
        wd = wxdp.tile([128, PIX], bf16, tag=f"wxd{gp}", name=f"wxd{gp}")
        nc.scalar.dma_start(out=wd, in_=wx_sb[gp])
        wxd[gp] = wd

    xt_tiles = {}
    wrep_tiles = {}
    wrepp_box = {}

    def load_xt(g):
        if g in xt_tiles:
            return xt_tiles[g]
        xt = xtp.tile([128, 4, PIX], bf16, tag="xt", name=f"xt{g}")
        for cpc in range(4):
            # partition = (cpk, t): channel c = cpc*64 + cpk*4 + g
            src = bass.AP(
                tensor=xin_base.tensor,
                offset=(cpc * 64 + g) * PIX,
                ap=[[4 * PIX, 16], [C * PIX, T], [1, PIX]],
            )
            nc.sync.dma_start(out=xt[:, cpc, :], in_=src)
        xt_tiles[g] = xt
        return xt

    def make_wrep(g, o):
        if (g, o) in wrep_tiles:
            return wrep_tiles.pop((g, o))
        rowbase = (g % 2) * 64 + o * 8
        wrep = wrepp_box["pool"].tile([128, PIX], bf16, tag="wrep", name="wrep")
        wsrc = bass.AP(
            tensor=wxd[g // 2].tensor,
            offset=wxd[g // 2].offset + rowbase * PIX,
            ap=[[0, 16], [PIX, 8], [1, PIX]],
        )
        nc.sync.dma_start(out=wrep, in_=wsrc)
        return wrep

    def phase2(g, pr2p, xobp):
        xt = load_xt(g)
        for o in range(TO):
            wrep = make_wrep(g, o)
            if o == TO - 2 and g < G - 1:
                # prefetch the next group's xt + first wreps during this
                # group's tail products (kills the g-transition DVE stall)
                load_xt(g + 1)
                for oo in range(2):
                    if (g + 1, oo) not in wrep_tiles:
                        wrep_tiles[(g + 1, oo)] = make_wrep(g + 1, oo)
            # for the very last (g, o) compute products chunk-wise so the
            # trailing MM/drain/DMA chain starts ~6us earlier
            fine = (g == 3 and o == TO - 1)
            # cpc-pair products in one DVE op: wrep broadcast over cpc
            # bufs=3: the product for (g,o) reuses (g,o-2)'s buffer, and the
            # trailing matmul stream occasionally runs ~1us behind -- a third
            # buffer absorbs that jitter (frames-era pools are closed here)
            pr4 = pr2p.tile([128, 4, PIX], bf16, tag="pr2", bufs=3, name="pr4")
            if not fine:
                # all 4 cpc products in a single DVE op: one instruction
                # overhead instead of two, wrep broadcast over cpc
                wb = wrep[:, :].unsqueeze(1).broadcast_to((128, 4, PIX))
                nc.vector.tensor_mul(pr4, xt, wb)
            xout = xobp.tile([128, PIX], bf16, tag="xout", name="xout")
            for c in range(NCH):
                cs, ce = c * CHN, (c + 1) * CHN
                if fine:
                    wbc = wrep[:, cs:ce].unsqueeze(1).broadcast_to((128, 4, CHN))
                    nc.vector.tensor_mul(
                        pr4[:, :, cs:ce], xt[:, :, cs:ce], wbc
                    )
                xop = psum.tile([128, CHN], fp32, tag=f"b{c}", name=f"xo{g}_{o}_{c}")
                # 4 col-tiled t-reductions run concurrently
                for cpc in range(4):
                    nc.tensor.matmul(
                        xop[32 * cpc:32 * cpc + 16, :],
                        to_sb[:, cpc, :],
                        pr4[:, cpc, cs:ce],
                        start=True,
                        stop=True,
                        tile_position=(0, 32 * cpc),
                    )
                if fine and c == NCH - 1:
                    # the very last drain on VectorE: it is idle after the
                    # final product, and this overlaps ScalarE's drain
                    # backlog so the last output wave fires ~0.6us earlier
                    nc.vector.tensor_copy(xout[:, cs:ce], xop)
                else:
                    nc.scalar.copy(xout[:, cs:ce], xop)
                if fine and c == 3:
                    # first output wave (chunks 0-3) ships while the tail
                    # chunks still compute; the final wave is then small
                    for cpc in range(4):
                        eng = (nc.sync, nc.scalar, nc.sync, nc.scalar)[cpc]
                        eng.dma_start(
                            out=out_r[o, g, cpc, :, 0:4 * CHN],
                            in_=xout[32 * cpc:32 * cpc + 16, 0:4 * CHN],
                        )
            lo = 4 * CHN if fine else 0
            for cpc in range(4):
                # keep the sync queue free for wrep/xt and the scalar
                # queue free for drains: triggers go to GpSimd's SWDGE.
                # last two (g,o): sync/scalar HWDGE only, so GpSimd's slow
                # final drain-wait covers long-completed DMAs and the
                # teardown path never waits on a stale SWDGE queue
                if g == G - 1 and o == TO - 1:
                    # three queues: DVE is past its port-locking 2-port ops
                    # here, so the gpsimd trigger generates unstalled
                    eng = (nc.sync, nc.scalar, nc.gpsimd, nc.sync)[cpc]
                elif g == G - 1 and o == TO - 2:
                    eng = (nc.sync, nc.scalar, nc.sync, nc.scalar)[cpc]
                else:
                    eng = nc.gpsimd
                eng.dma_start(
                    out=out_r[o, g, cpc, :, lo:],
                    in_=xout[32 * cpc:32 * cpc + 16, lo:],
                )

    with tc.tile_pool(name="wrep", bufs=4) as wrepp:
        wrepp_box["pool"] = wrepp
        with tc.tile_pool(name="frames", bufs=1) as frames, \
             tc.tile_pool(name="stage", bufs=4) as stage, \
             tc.tile_pool(name="prods", bufs=2) as prods:
            frames_pools["pools"] = (frames, stage, prods)
            corr_phase(0)
            conv_phase(0)
            # prefetch phase2(g0)'s inputs so they transfer during ct1
            load_xt(0)
            for o in range(3):
                wrep_tiles[(0, o)] = make_wrep(0, o)
            corr_phase(1)
            # conv(1) right away: its wx activations land on the scalar
            # queue ahead of phase2's xout drains, and wxd[1] is ready
            # long before the g=2/3 wrep prefetches need it
            conv_phase(1)
        with tc.tile_pool(name="pr2", bufs=3) as pr2p, \
             tc.tile_pool(name="xob", bufs=3) as xobp:
            phase2(0, pr2p, xobp)
            phase2(1, pr2p, xobp)
            phase2(2, pr2p, xobp)
            phase2(3, pr2p, xobp)


def _get_module(loop_k=1):
    key = f"nc{loop_k}"
    if key not in _CACHE:
        _CACHE[key] = _build_module(loop_k)
    return _CACHE[key]


def _consts(conv_w, conv_b):
    conv_w = np.asarray(conv_w, np.float32)
    conv_b = np.asarray(conv_b, np.float32)
    # block-diagonal fused conv weights per group-pair, k-rows permuted to
    # the col-tiled corr layout: corr index i, channel-half gh lives at
    # k-row 32*(i%4) + 2*(i//4) + gh.  Bias (+1.0 residual when t==o)
    # applied at the PSUM drain as a per-partition activation bias (bvec).
    wf2 = np.zeros((128, 2, 128), np.float32)
    bvec = np.zeros((128, 2), np.float32)
    for gp in range(2):
        for gh in range(2):
            g = gp * 2 + gh
            for o in range(TO):
                for t in range(T):
                    m = gh * 64 + o * 8 + t
                    for i in range(63):
                        wf2[_corr_row(i, gh), gp, m] = conv_w[g, o * 8 + t, i]
                    bvec[m, gp] = conv_b[g, o * 8 + t] + (1.0 if t == o else 0.0)

    # shifted ones matrix for the col-tiled channel-sum:
    # lhsT for product r is bm[:, 62-2q : 94-2q] (q = r//4) and must have
    # ones at [0:64, m=2q] and [64:128, m=2q+1]
    bm = np.zeros((128, 96), np.float32)
    bm[0:64, 62] = 1.0
    bm[64:128, 63] = 1.0

    # t-reduce ones per col group: to[p=(cpk,t), cpc, m] = 1 iff m == cpk
    to = np.zeros((128, 4, 16), np.float32)
    for cpc in range(4):
        for cpk in range(16):
            to[cpk * 8:(cpk + 1) * 8, cpc, cpk] = 1.0

    return (
        wf2.astype(ml_dtypes.bfloat16),
        bm.astype(ml_dtypes.bfloat16),
        to.astype(ml_dtypes.bfloat16),
        bvec,
    )


def kernel(x, conv_w, conv_b):
    from concourse.bass_utils import run_bass_kernel_spmd

    nc = _get_module()
    wf, bm, to, bv = _consts(conv_w, conv_b)
    x = np.asarray(x, np.float32).astype(ml_dtypes.bfloat16)
    x8 = np.ascontiguousarray(x.reshape(NCORES, T, C, H, W))
    in_maps = [
        {
            "xin": np.ascontiguousarray(x8[i]),
            "wf2": wf,
            "bm2": bm,
            "tones": to,
            "bvec": bv,
        }
        for i in range(NCORES)
    ]
    res = run_bass_kernel_spmd(nc, in_maps, core_ids=list(range(NCORES)))
    outs = [r["out"] for r in res.results]
    return np.concatenate(outs, axis=0).astype(np.float32)



# revision 76
# speedup vs baseline: 1.0205x; 1.0205x over previous
"""CorrelationFusion Trainium2 kernel.

Per-clip math (T=8 frames, G=4 groups, 3x3 correlation window):
  corr[g, tt*9+ij, p] = sum_cp x[tt, g*64+cp, p] * xpad[tt+1, g*64+cp, p+d(ij)]
  wx[g, o*8+t, p]     = sum_i conv_w[g, o*8+t, i]*corr[g, i, p] + conv_b[g, o*8+t]
  out[o, g*64+cp, p]  = sum_t wx'[g, o*8+t, p] * x[t, cp*4+g, p]
  (wx' = wx + 1 on the t==o rows -- the residual folded into the conv bias)

Mapping:
  - per-pixel products on VectorE in bf16 (2x mode), channels on partitions;
    corr products merged into di-trios (one DVE op per (tt, dj) via an
    overlapping 3-row-window AP), phase2 products one op per (g, o)
  - partition reductions via TensorE matmuls, 4-way COLUMN-TILED
    (tile_position col groups) so 4 small-M matmuls run concurrently
  - the 1x1 grouped conv is a block-diagonal matmul over the 63 corr rows
    (k-rows permuted to match the col-tiled corr psum layout)
  - frames DMA'd contiguously into flat staging tiles ('a' operands and the
    center 'b' read those directly); padded P/S copies + edge replication
    on ScalarE (strided DMA writes measured 4x slower -- don't)
  - wx rows replicated into the (cp16, t8)-interleaved layout via a
    broadcast-read DMA from a DRAM bounce buffer
  - corr / conv weights / output in bf16 (rel-err budget has 4x margin);
    host converts the bf16 output back to fp32
  - data-parallel over the 8 clips: one clip per NeuronCore
"""

import numpy as np
import ml_dtypes

T = 8
TO = 8
G = 4
CPG = 64
C = 256
H = 56
W = 56
PIX = H * W
NCORES = 8
PH = 58   # padded tile rows
PW = 60   # padded tile cols (extra pad for 4B alignment of bf16 rows)
NCH = 7   # pixel chunks per image
CHW = 8   # rows per chunk
CHN = CHW * W  # 448 pixels per chunk
_CACHE = {}


def _corr_row(i, gh):
    """psum/partition row of corr index i (0..62) for channel-half gh."""
    return 32 * (i % 4) + 2 * (i // 4) + gh


def _exec_seq():
    """corr product execution order: tt=0 specially ordered for the head,
    tt>=1 grouped in di-trios per dj."""
    seq = [4, 1, 7, 0, 3, 6, 2, 5, 8]
    for tt in range(1, T - 1):
        for dj in (-1, 0, 1):
            seq += [tt * 9 + k * 3 + (dj + 1) for k in range(3)]
    return seq


_SEQ = _exec_seq()
_FIRSTS, _LASTS = {}, {}
for _r in _SEQ:
    _FIRSTS.setdefault(_r % 4, _r)
    _LASTS[_r % 4] = _r
FIRST_RS = frozenset(_FIRSTS.values())
LAST_RS = frozenset(_LASTS.values())


def _build_module(loop_k=1):
    import contextlib

    import concourse.bass as bass
    import concourse.bacc as bacc
    import concourse.mybir as mybir
    import concourse.tile as tile

    fp32 = mybir.dt.float32
    bf16 = mybir.dt.bfloat16

    nc = bacc.Bacc(name="corrfusion")
    xin = nc.dram_tensor("xin", [T, C, H, W], bf16, kind="ExternalInput")
    wf2 = nc.dram_tensor("wf2", [128, 2, 128], bf16, kind="ExternalInput")
    bm2 = nc.dram_tensor("bm2", [128, 96], bf16, kind="ExternalInput")
    tones = nc.dram_tensor("tones", [128, 4, 16], bf16, kind="ExternalInput")
    bvec = nc.dram_tensor("bvec", [128, 2], fp32, kind="ExternalInput")
    out = nc.dram_tensor("out", [TO, C, H, W], bf16, kind="ExternalOutput")

    xin_base = xin[:, :, :, :]                                 # base AP for manual APs
    out_r = out.rearrange("o (g cpc k) h w -> o g cpc k (h w)", g=4, cpc=4, k=16)

    with tile.TileContext(nc) as tc:
        with tc.tile_pool(name="consts", bufs=1) as consts, \
             tc.tile_pool(name="corrbuf", bufs=1) as corrbuf, \
             tc.tile_pool(name="xt", bufs=2) as xtp, \
             tc.tile_pool(name="wxdp", bufs=1, space="DRAM") as wxdp, \
             tc.tile_pool(name="psum", bufs=1, space="PSUM") as psum:

            # consts on the gpsimd queue: scalar must be free for the
            # first frame-pad copies, sync for the first stage loads
            wf_sb = consts.tile([128, 2, 128], bf16)
            nc.gpsimd.dma_start(out=wf_sb, in_=wf2[:, :, :])
            bm_sb = consts.tile([128, 96], bf16)
            nc.gpsimd.dma_start(out=bm_sb, in_=bm2[:, :])
            to_sb = consts.tile([128, 4, 16], bf16)
            nc.gpsimd.dma_start(out=to_sb, in_=tones[:, :, :])
            bv_sb = consts.tile([128, 2], fp32)
            nc.gpsimd.dma_start(out=bv_sb, in_=bvec[:, :])

            # shared tag: corr_sb[0] is dead once conv(0)'s matmuls have
            # read it, well before corr(1)'s drains write the buffer
            corr_sb = [
                corrbuf.tile([128, PIX], bf16, tag="corr", name=f"corr{i}")
                for i in range(2)
            ]
            # one shared staging buffer: wx_sb[gp] is dead once the wxd
            # bounce DMA has read it, so the two conv phases can share
            wx_sb = [
                corrbuf.tile([128, PIX], bf16, tag="wx", name=f"wx{i}")
                for i in range(2)
            ]
            loop_cm = (
                tc.For_i(0, loop_k, 1) if loop_k > 1 else contextlib.nullcontext()
            )
            with loop_cm:
                _build_body(nc, tc, tile, bass, mybir, fp32, bf16, xin, xin_base,
                            out_r, wf_sb, bm_sb, to_sb, bv_sb, corr_sb, wx_sb,
                            xtp, wxdp, psum)
    nc.compile()
    return nc


def _build_body(nc, tc, tile, bass, mybir, fp32, bf16, xin, xin_base, out_r,
                wf_sb, bm_sb, to_sb, bv_sb, corr_sb, wx_sb, xtp, wxdp, psum):
    # rows 126/127 (unused by the col-tiled corr layout) must read as zeros
    # in the conv matmul; memset the whole 32-row group (engines need a
    # 32-aligned partition base), the drains overwrite rows 96..125
    # (gpsimd: vector/scalar must stay clear for the first frame work)
    for i in range(2):
        nc.gpsimd.memset(corr_sb[i][96:128, :], 0.0)

    wxd = [None, None]

    frames_pools = {}  # set by the caller: frames, stage, prods
    ptiles = {0: {}, 1: {}}
    stiles = {0: {}, 1: {}}
    sgtiles = {0: {}, 1: {}}

    def load_stg(ct, t, split=False):
        frames, stage, prods = frames_pools["pools"]
        cs = ct * 128
        stg = stage.tile([128, PIX], bf16, tag="fstage", name=f"stg{ct}_{t}")
        if split:
            # pixel-half DMAs: the first (center) product can start on the
            # first half while the second is still in flight
            hp = 4 * CHN
            nc.sync.dma_start(out=stg[:, 0:hp], in_=xin[t, cs:cs + 128, 0:32, :])
            nc.sync.dma_start(out=stg[:, hp:], in_=xin[t, cs:cs + 128, 32:56, :])
        else:
            nc.sync.dma_start(out=stg, in_=xin[t, cs:cs + 128, :, :])
        sgtiles[ct][t] = stg
        return stg

    def load_frame(ct, t, warmup=False, stg=None):
        frames, stage, prods = frames_pools["pools"]
        ptile, stile, sgtile = ptiles[ct], stiles[ct], sgtiles[ct]
        # contiguous DMA into a staging tile; 'a' operands and the center
        # (di=0,dj=0) 'b' operand read the flat staging tile directly, so
        # the padded-copy latency is off the product critical path
        if stg is None:
            stg = load_stg(ct, t)
        stg3 = stg.rearrange("p (h w) -> p h w", h=H)
        if t > 0:
            # frame 0 is only ever the 'a' operand: no padded tiles needed.
            # P serves the di-shifted dj=0 'b' reads (rows padded).
            # warmup (very first pair): vector is idle until the first
            # stage tiles land, so it carries the P work itself
            pcopy = nc.vector.tensor_copy if warmup else nc.scalar.copy
            P = frames.tile([128, PH, PW], bf16, tag=f"P{t % 4}", name=f"P{ct}_{t}")
            if warmup:
                # row-halves matching the split head DMAs: the first copy
                # starts as soon as the first half-DMA lands
                pcopy(P[:, 1:33, 2:58], stg3[:, 0:32, :])
                pcopy(P[:, 33:57, 2:58], stg3[:, 32:56, :])
            else:
                pcopy(P[:, 1:57, 2:58], stg3)
            pcopy(P[:, 0:1, 2:58], P[:, 1:2, 2:58])
            pcopy(P[:, 57:58, 2:58], P[:, 56:57, 2:58])
            ptile[t] = P
            # S always on scalar: with the split head DMAs its chain
            # finishes before the first S-dependent trio needs it, and the
            # vector queue keeps those ~2us for products
            S = frames.tile([128, PH, PW], bf16, tag=f"S{t % 4}", name=f"S{ct}_{t}")
            scopy = nc.scalar.copy
            if warmup:
                scopy(S[:, 1:33, 1:57], stg3[:, 0:32, :])
                scopy(S[:, 33:57, 1:57], stg3[:, 32:56, :])
            else:
                scopy(S[:, 1:57, 1:57], stg3)
            # S is read at cols 0:56 / 2:58 (dj=-1/+1): col pads 0 and 57
            scopy(S[:, 1:57, 0:1], S[:, 1:57, 1:2])
            scopy(S[:, 1:57, 57:58], S[:, 1:57, 56:57])
            scopy(S[:, 0:1, 0:58], S[:, 1:2, 0:58])
            scopy(S[:, 57:58, 0:58], S[:, 56:57, 0:58])
            stile[t] = S

    def corr_phase(ct):
        if True:
            frames, stage, prods = frames_pools["pools"]
            cps = [
                psum.tile([128, CHN], fp32, tag=f"b{c}", name=f"cps{ct}_{c}")
                for c in range(NCH)
            ]
            ptile = ptiles[ct]
            stile = stiles[ct]
            sgtile = sgtiles[ct]

            if ct == 0 and 0 not in sgtile and 1 not in sgtile:
                # both head frames, half-DMAs interleaved across frames so
                # the first halves of BOTH land first and the center
                # product starts at ~half the total flight time.
                # (padded copies are emitted after the first product.)
                frames_, stage_, _ = frames_pools["pools"]
                hp = 4 * CHN
                stgs = []
                for t in (0, 1):
                    stgs.append(
                        stage_.tile([128, PIX], bf16, tag="fstage", name=f"stg0_{t}")
                    )
                    sgtile[t] = stgs[t]
                for half in range(2):
                    s, e = (0, hp) if half == 0 else (hp, PIX)
                    r0, r1 = (0, 32) if half == 0 else (32, 56)
                    for t in (0, 1):
                        nc.sync.dma_start(
                            out=stgs[t][:, s:e], in_=xin[t, 0:128, r0:r1, :]
                        )
            if 0 not in sgtile:
                load_frame(ct, 0)
            if 1 not in sgtile:
                load_frame(ct, 1)
            def emit_reduce(r, pr2d):
                # col-tiled reduction: product r -> col group r%4,
                # rows 2*(r//4)+gh; lhsT = 32-col slice of the shifted
                # ones matrix.  start/stop from the precomputed execution
                # order (trio grouping reorders products within a tt)
                cg = r % 4
                q = r // 4
                lhsT = bm_sb[:, 62 - 2 * q:94 - 2 * q]
                for c in range(NCH):
                    nc.tensor.matmul(
                        cps[c][32 * cg:32 * cg + 32, :],
                        lhsT,
                        pr2d[:, c * CHN:(c + 1) * CHN],
                        start=(r in FIRST_RS),
                        stop=(r in LAST_RS),
                        tile_position=(0, 32 * cg),
                    )

            def b_win(tt, dj, n):
                # overlapping di-window AP: [128, n, 56, 56], di stride =
                # one padded row (60); n=3 covers di in {-1,0,1}, n=2 the
                # duo {-1,+1} (stride 2 rows)
                tile_ap = ptile[tt + 1] if dj == 0 else stile[tt + 1]
                col = 2 if dj >= 0 else 0
                return bass.AP(
                    tensor=tile_ap.tensor,
                    offset=tile_ap.offset + col,
                    ap=[[PH * PW, 128], [PW * (4 - n), n], [PW, 56], [1, 56]],
                )

            for tt in range(T - 1):
                if tt + 2 < T and not (ct == 0 and tt == 0):
                    load_frame(ct, tt + 2)
                if ct == 0 and tt == T - 2:
                    # preload next half's first frames while this half's
                    # tail products still run (keeps the scalar queue from
                    # serializing drains ahead of them)
                    load_frame(1, 0)
                    load_frame(1, 1)
                a3 = sgtile[tt].rearrange("p (h w) -> p h w", h=H)
                if tt == 0:
                    # center product split in pixel halves (reads only the
                    # flat half-loaded stage tiles) leads the vector queue
                    prc = prods.tile([128, 3, PIX], bf16, tag="ptrio", name="prc")
                    HP = 4 * CHN
                    nc.vector.tensor_mul(
                        prc[:, 0, 0:HP], sgtile[0][:, 0:HP], sgtile[1][:, 0:HP]
                    )
                    if ct == 0 and 1 not in ptile:
                        # first padded tiles right behind the first product;
                        # frame 2 deferred so the scheduler can't slot its
                        # big copies ahead of S1's pads on the scalar queue
                        load_frame(0, 1, warmup=True, stg=sgtile[1])
                    nc.vector.tensor_mul(
                        prc[:, 0, HP:], sgtile[0][:, HP:], sgtile[1][:, HP:]
                    )
                    emit_reduce(tt * 9 + 4, prc[:, 0, :])
                    # dj=0 duo (di = -1,+1), then the two S trios
                    duo = prc[:, 1:3, :].rearrange("p d (h w) -> p d h w", h=H)
                    a2 = a3.unsqueeze(1).broadcast_to((128, 2, H, W))
                    nc.vector.tensor_mul(duo, a2, b_win(tt, 0, 2))
                    emit_reduce(tt * 9 + 1, prc[:, 1, :])
                    emit_reduce(tt * 9 + 7, prc[:, 2, :])
                    dj_iter = (-1, 1)
                else:
                    dj_iter = (-1, 0, 1)
                a3b = a3.unsqueeze(1).broadcast_to((128, 3, H, W))
                for dj in dj_iter:
                    pr = prods.tile([128, 3, PIX], bf16, tag="ptrio", name="pr")
                    pr4d = pr.rearrange("p d (h w) -> p d h w", h=H)
                    nc.vector.tensor_mul(pr4d, a3b, b_win(tt, dj, 3))
                    for k in range(3):
                        emit_reduce(tt * 9 + k * 3 + (dj + 1), pr[:, k, :])
                if ct == 0 and tt == 0 and 2 not in sgtile:
                    load_frame(0, 2)
            for c in range(NCH):
                nc.scalar.copy(
                    corr_sb[ct][0:126, c * CHN:(c + 1) * CHN],
                    cps[c][0:126, :],
                )

    def conv_phase(gp):
        # grouped 1x1 conv (+bias +residual); bounce wx to DRAM so the
        # per-(o,g) replication is one broadcast-read DMA
        for c in range(NCH):
            wpp = psum.tile([128, CHN], fp32, tag="wp", name=f"wpp{gp}_{c}")
            nc.tensor.matmul(
                wpp,
                wf_sb[:, gp, :],
                corr_sb[gp][:, c * CHN:(c + 1) * CHN],
                start=True,
                stop=True,
            )
            nc.scalar.activation(
                wx_sb[gp][:, c * CHN:(c + 1) * CHN],
                wpp,
                mybir.ActivationFunctionType.Identity,
                bias=bv_sb[:, gp:gp + 1],
                scale=1.0,
            )
        # trigger on the scalar queue: its dep (the wx activations) sits
        # right before it there, so it never blocks a queue head
        wd = wxdp.tile([128, PIX], bf16, tag=f"wxd{gp}", name=f"wxd{gp}")
        nc.scalar.dma_start(out=wd, in_=wx_sb[gp])
        wxd[gp] = wd

    xt_tiles = {}
    wrep_tiles = {}
    wrepp_box = {}

    def load_xt(g):
        if g in xt_tiles:
            return xt_tiles[g]
        xt = xtp.tile([128, 4, PIX], bf16, tag="xt", name=f"xt{g}")
        for cpc in range(4):
            # partition = (cpk, t): channel c = cpc*64 + cpk*4 + g
            src = bass.AP(
                tensor=xin_base.tensor,
                offset=(cpc * 64 + g) * PIX,
                ap=[[4 * PIX, 16], [C * PIX, T], [1, PIX]],
            )
            nc.sync.dma_start(out=xt[:, cpc, :], in_=src)
        xt_tiles[g] = xt
        return xt

    def make_wrep(g, o):
        if (g, o) in wrep_tiles:
            return wrep_tiles.pop((g, o))
        rowbase = (g % 2) * 64 + o * 8
        wrep = wrepp_box["pool"].tile([128, PIX], bf16, tag="wrep", name="wrep")
        wsrc = bass.AP(
            tensor=wxd[g // 2].tensor,
            offset=wxd[g // 2].offset + rowbase * PIX,
            ap=[[0, 16], [PIX, 8], [1, PIX]],
        )
        nc.sync.dma_start(out=wrep, in_=wsrc)
        return wrep

    def phase2(g, pr2p, xobp):
        xt = load_xt(g)
        for o in range(TO):
            wrep = make_wrep(g, o)
            if o == TO - 2 and g < G - 1:
                # prefetch the next group's xt + first wreps during this
                # group's tail products (kills the g-transition DVE stall)
                load_xt(g + 1)
                for oo in range(2):
                    if (g + 1, oo) not in wrep_tiles:
                        wrep_tiles[(g + 1, oo)] = make_wrep(g + 1, oo)
            # for the very last (g, o) compute products chunk-wise so the
            # trailing MM/drain/DMA chain starts ~6us earlier
            fine = (g == 3 and o == TO - 1)
            # cpc-pair products in one DVE op: wrep broadcast over cpc
            # bufs=3: the product for (g,o) reuses (g,o-2)'s buffer, and the
            # trailing matmul stream occasionally runs ~1us behind -- a third
            # buffer absorbs that jitter (frames-era pools are closed here)
            pr4 = pr2p.tile([128, 4, PIX], bf16, tag="pr2", bufs=3, name="pr4")
            if not fine:
                # all 4 cpc products in a single DVE op: one instruction
                # overhead instead of two, wrep broadcast over cpc
                wb = wrep[:, :].unsqueeze(1).broadcast_to((128, 4, PIX))
                nc.vector.tensor_mul(pr4, xt, wb)
            xout = xobp.tile([128, PIX], bf16, tag="xout", name="xout")
            for c in range(NCH):
                cs, ce = c * CHN, (c + 1) * CHN
                if fine:
                    wbc = wrep[:, cs:ce].unsqueeze(1).broadcast_to((128, 4, CHN))
                    nc.vector.tensor_mul(
                        pr4[:, :, cs:ce], xt[:, :, cs:ce], wbc
                    )
                xop = psum.tile([128, CHN], fp32, tag=f"b{c}", name=f"xo{g}_{o}_{c}")
                # 4 col-tiled t-reductions run concurrently
                for cpc in range(4):
                    nc.tensor.matmul(
                        xop[32 * cpc:32 * cpc + 16, :],
                        to_sb[:, cpc, :],
                        pr4[:, cpc, cs:ce],
                        start=True,
                        stop=True,
                        tile_position=(0, 32 * cpc),
                    )
                if fine and c == NCH - 1:
                    # the very last drain on VectorE: it is idle after the
                    # final product, and this overlaps ScalarE's drain
                    # backlog so the last output wave fires ~0.6us earlier
                    nc.vector.tensor_copy(xout[:, cs:ce], xop)
                else:
                    nc.scalar.copy(xout[:, cs:ce], xop)
                if fine and c == 3:
                    # first output wave (chunks 0-3) ships while the tail
                    # chunks still compute; the final wave is then small
                    for cpc in range(4):
                        eng = (nc.sync, nc.scalar, nc.sync, nc.scalar)[cpc]
                        eng.dma_start(
                            out=out_r[o, g, cpc, :, 0:4 * CHN],
                            in_=xout[32 * cpc:32 * cpc + 16, 0:4 * CHN],
                        )
            lo = 4 * CHN if fine else 0
            for cpc in range(4):
                # keep the sync queue free for wrep/xt and the scalar
                # queue free for drains: triggers go to GpSimd's SWDGE.
                # last two (g,o): sync/scalar HWDGE only, so GpSimd's slow
                # final drain-wait covers long-completed DMAs and the
                # teardown path never waits on a stale SWDGE queue
                if g == G - 1 and o >= TO - 2:
                    eng = (nc.sync, nc.scalar, nc.sync, nc.scalar)[cpc]
                else:
                    eng = nc.gpsimd
                eng.dma_start(
                    out=out_r[o, g, cpc, :, lo:],
                    in_=xout[32 * cpc:32 * cpc + 16, lo:],
                )

    with tc.tile_pool(name="wrep", bufs=4) as wrepp:
        wrepp_box["pool"] = wrepp
        with tc.tile_pool(name="frames", bufs=1) as frames, \
             tc.tile_pool(name="stage", bufs=4) as stage, \
             tc.tile_pool(name="prods", bufs=2) as prods:
            frames_pools["pools"] = (frames, stage, prods)
            corr_phase(0)
            conv_phase(0)
            # prefetch phase2(g0)'s inputs so they transfer during ct1
            load_xt(0)
            for o in range(3):
                wrep_tiles[(0, o)] = make_wrep(0, o)
            corr_phase(1)
            # conv(1) right away: its wx activations land on the scalar
            # queue ahead of phase2's xout drains, and wxd[1] is ready
            # long before the g=2/3 wrep prefetches need it
            conv_phase(1)
        with tc.tile_pool(name="pr2", bufs=3) as pr2p, \
             tc.tile_pool(name="xob", bufs=3) as xobp:
            phase2(0, pr2p, xobp)
            phase2(1, pr2p, xobp)
            phase2(2, pr2p, xobp)
            phase2(3, pr2p, xobp)


def _get_module(loop_k=1):
    key = f"nc{loop_k}"
    if key not in _CACHE:
        _CACHE[key] = _build_module(loop_k)
    return _CACHE[key]


def _consts(conv_w, conv_b):
    conv_w = np.asarray(conv_w, np.float32)
    conv_b = np.asarray(conv_b, np.float32)
    # block-diagonal fused conv weights per group-pair, k-rows permuted to
    # the col-tiled corr layout: corr index i, channel-half gh lives at
    # k-row 32*(i%4) + 2*(i//4) + gh.  Bias (+1.0 residual when t==o)
    # applied at the PSUM drain as a per-partition activation bias (bvec).
    wf2 = np.zeros((128, 2, 128), np.float32)
    bvec = np.zeros((128, 2), np.float32)
    for gp in range(2):
        for gh in range(2):
            g = gp * 2 + gh
            for o in range(TO):
                for t in range(T):
                    m = gh * 64 + o * 8 + t
                    for i in range(63):
                        wf2[_corr_row(i, gh), gp, m] = conv_w[g, o * 8 + t, i]
                    bvec[m, gp] = conv_b[g, o * 8 + t] + (1.0 if t == o else 0.0)

    # shifted ones matrix for the col-tiled channel-sum:
    # lhsT for product r is bm[:, 62-2q : 94-2q] (q = r//4) and must have
    # ones at [0:64, m=2q] and [64:128, m=2q+1]
    bm = np.zeros((128, 96), np.float32)
    bm[0:64, 62] = 1.0
    bm[64:128, 63] = 1.0

    # t-reduce ones per col group: to[p=(cpk,t), cpc, m] = 1 iff m == cpk
    to = np.zeros((128, 4, 16), np.float32)
    for cpc in range(4):
        for cpk in range(16):
            to[cpk * 8:(cpk + 1) * 8, cpc, cpk] = 1.0

    return (
        wf2.astype(ml_dtypes.bfloat16),
        bm.astype(ml_dtypes.bfloat16),
        to.astype(ml_dtypes.bfloat16),
        bvec,
    )


def kernel(x, conv_w, conv_b):
    from concourse.bass_utils import run_bass_kernel_spmd

    nc = _get_module()
    wf, bm, to, bv = _consts(conv_w, conv_b)
    x = np.asarray(x, np.float32).astype(ml_dtypes.bfloat16)
    x8 = np.ascontiguousarray(x.reshape(NCORES, T, C, H, W))
    in_maps = [
        {
            "xin": np.ascontiguousarray(x8[i]),
            "wf2": wf,
            "bm2": bm,
            "tones": to,
            "bvec": bv,
        }
        for i in range(NCORES)
    ]
    res = run_bass_kernel_spmd(nc, in_maps, core_ids=list(range(NCORES)))
    outs = [r["out"] for r in res.results]
    return np.concatenate(outs, axis=0).astype(np.float32)

